# revision 31
# baseline (speedup 1.0000x reference)
"""CRF partition function (neg log partition) on 8 Trainium2 NeuronCores.

Algorithm: rank-1 chunked scan ("warmup stitch").
  In prob space the recurrence is p_t = p_{t-1} @ (E D_t), E = exp(log_transition)
  (row-stochastic), D_t = diag(exp(obs_t)). Products of positive matrices contract
  to rank-1 at exp rate (~1 decade/step for this E), so the T=4096 serial scan
  splits into C = T/L independent chunks of L steps: each chunk is run from a
  probe vector with W warmup steps that recover the true state *direction*;
  per-chunk scale corrections are ratios of vector sums, stitched on host in
  fp64. At L=8, W=1: rel err ~1e-4 on hardware incl. bf16 quantization
  (tolerance is 2e-2).

Device layout (per core, 8 batches): chains n = c*8 + b, N = C*8 columns.
G = L+W "slices" g: X[128 states, N] bf16; step: X = (E^T X) * e_slice[g] with
E stationary on PE (matmul, 512-col chunks = one PSUM bank each), elementwise
mul on DVE straight from PSUM (1x mode; an ACT-evacuate + DVE-2x hybrid
measures *slower* here because the extra PE->ACT->DVE hop lengthens the
per-slice dependency cycle). e-slices are host-precomputed exp(obs - 0.5) in
bf16, pre-transposed/padded so the device does only: DMA slice -> matmul ->
multiply. Chunk-0 chains re-init with the exact p_0 at g=W-1; the last chunk
consumes W pad columns e=1 (exact: E row-stochastic). Host stitch removes the
0.5 bias.

Tuning (measured via hardware-loop marginals, see bench.py): L=8/W=1 ->
G=9 slices of N=4096 columns; one [128,4096] f32 PSUM tile (all 8 banks,
single-buffered, sub-region WAR deps rotate it). Per slice the 8 512-col
regions split across engines: 5 DVE tensor_mul direct from PSUM + 3 regions
ACT-copy (PSUM->bf16) then GPSIMD tensor_mul, which takes the multiply off
the DVE critical path: ~54us/core vs ~65us all-DVE and ~68-70us for the
previous L=64/W=8/N=512 layout.
"""

import contextlib

import numpy as np
import ml_dtypes

import concourse.bacc as bacc
import concourse.mybir as mybir
from concourse.tile import TileContext
from concourse.bass_utils import run_bass_kernel_spmd

bf16 = ml_dtypes.bfloat16

B, T, S = 64, 4096, 128
BETA = 0.5
NCORES = 8
BPC = B // NCORES     # 8 batches per core
GRP = 2               # ping-pong groups for PE/DVE overlap

L, W = 8, 1           # chunk length, warmup (see configure())
C = T // L
G = L + W
N = C * BPC

# Engine split of the per-slice elementwise multiply (see _build_fast_program):
# nA*512 cols minus nGp*512 go DVE-direct-from-PSUM (in aSplit instructions),
# nGp*512 cols go ACT-copy + GPSIMD-multiply, N - nA*512 cols go ACT-copy +
# DVE-2x-bf16 in mB instructions.
FAST_CFG = dict(nA=8, mB=0, aSplit=5, nGp=3)


def configure(l, w):
    """Set chunk length / warmup; recomputes C (chunks), G (slices), N (chains/core)."""
    global L, W, C, G, N
    L, W = l, w
    C = T // L
    G = L + W
    N = C * BPC


def _build_device_program(grp=GRP, esbufs=6, xbufs=3, psbufs=4, mode="dve", ksplit=256,
                          repeats=1, loop_trips=0):
    """mode="dve": both groups evacuated by DVE tensor_mul straight from PSUM.
    mode="split": group A (ksplit cols) evacuated by ACT copy (PSUM->bf16 SBUF)
    then multiplied on DVE in 2x bf16 mode; group B by DVE direct from PSUM.
    repeats>1 re-runs the whole slice loop (for marginal timing; outputs are
    identical each repeat).
    Bacc (not raw Bass): its .compile() runs generate_event_semaphores, which
    legalizes multi-wait instructions for walrus codegen."""
    nc = bacc.Bacc("TRN2", target_bir_lowering=False)
    es_d = nc.dram_tensor("eslices", [G, S, N], mybir.dt.bfloat16, kind="ExternalInput")
    e0_d = nc.dram_tensor("e0", [S, BPC], mybir.dt.bfloat16, kind="ExternalInput")
    ew_d = nc.dram_tensor("ew", [S, S], mybir.dt.bfloat16, kind="ExternalInput")
    wout_d = nc.dram_tensor("wout", [S, N], mybir.dt.bfloat16, kind="ExternalOutput")
    yout_d = nc.dram_tensor("yout", [S, N], mybir.dt.bfloat16, kind="ExternalOutput")

    if mode == "split":
        groups = [(slice(0, ksplit), "act"), (slice(ksplit, N), "dve")]
    else:
        ng = N // grp
        groups = [(slice(h * ng, (h + 1) * ng), "dve") for h in range(grp)]

    with TileContext(nc) as tc:
        with (
            tc.tile_pool(name="const", bufs=1) as cpool,
            tc.tile_pool(name="es", bufs=esbufs) as espool,
            tc.tile_pool(name="state", bufs=xbufs) as xpool,
            tc.tile_pool(name="ev", bufs=3) as evpool,
            tc.tile_pool(name="ps", bufs=psbufs, space="PSUM") as ppool,
        ):
            E_sb = cpool.tile([S, S], mybir.dt.bfloat16)
            nc.sync.dma_start(out=E_sb[:], in_=ew_d[:])

            loop = tc.For_i(0, loop_trips) if loop_trips else contextlib.nullcontext()
            with loop:
              for _r in range(repeats):
                X = xpool.tile([S, N], mybir.dt.bfloat16)
                nc.vector.memset(X[:], 1.0)

                for g in range(G):
                    es = espool.tile([S, N], mybir.dt.bfloat16)
                    nc.sync.dma_start(out=es[:], in_=es_d[g])
                    Xn = xpool.tile([S, N], mybir.dt.bfloat16)
                    for sl, how in groups:
                        w = sl.stop - sl.start
                        ps = ppool.tile([S, w], mybir.dt.float32, tag=f"ps{how}{w}")
                        nc.tensor.matmul(
                            out=ps[:], lhsT=E_sb[:], rhs=X[:, sl], start=True, stop=True
                        )
                        if how == "act":
                            ev = evpool.tile([S, w], mybir.dt.bfloat16)
                            nc.scalar.copy(out=ev[:], in_=ps[:])
                            nc.vector.tensor_mul(out=Xn[:, sl], in0=ev[:], in1=es[:, sl])
                        else:
                            nc.vector.tensor_mul(out=Xn[:, sl], in0=ps[:], in1=es[:, sl])
                    if g == W - 1:
                        # snapshot w (pre-init state) and drop in exact chunk-0 init
                        nc.sync.dma_start(out=wout_d[:], in_=Xn[:])
                        nc.sync.dma_start(out=Xn[:, 0:BPC], in_=e0_d[:])
                    X = Xn
                nc.sync.dma_start(out=yout_d[:], in_=X[:])
    nc.compile()
    return nc


def _build_parts_program(repeats=1, esbufs=4, do_dma=True, do_mm=False, loop_trips=0):
    """Calibration: es DMA stream alone (do_dma) and/or the matmul stream alone
    (do_mm, reading a constant X so no elementwise is needed)."""
    nc = bacc.Bacc("TRN2", target_bir_lowering=False)
    es_d = nc.dram_tensor("eslices", [G, S, N], mybir.dt.bfloat16, kind="ExternalInput")
    e0_d = nc.dram_tensor("e0", [S, BPC], mybir.dt.bfloat16, kind="ExternalInput")
    ew_d = nc.dram_tensor("ew", [S, S], mybir.dt.bfloat16, kind="ExternalInput")
    wout_d = nc.dram_tensor("wout", [S, N], mybir.dt.bfloat16, kind="ExternalOutput")
    yout_d = nc.dram_tensor("yout", [S, N], mybir.dt.bfloat16, kind="ExternalOutput")
    nch = N // 512
    with TileContext(nc) as tc:
        with (
            tc.tile_pool(name="const", bufs=1) as cpool,
            tc.tile_pool(name="es", bufs=esbufs) as espool,
            tc.tile_pool(name="ps", bufs=2, space="PSUM") as ppool,
        ):
            E_sb = cpool.tile([S, S], mybir.dt.bfloat16)
            nc.sync.dma_start(out=E_sb[:], in_=ew_d[:])
            X = cpool.tile([S, N], mybir.dt.bfloat16)
            nc.vector.memset(X[:], 1.0)
            loop = tc.For_i(0, loop_trips) if loop_trips else contextlib.nullcontext()
            with loop:
              for _r in range(repeats):
                for g in range(G):
                    if do_dma:
                        es = espool.tile([S, N], mybir.dt.bfloat16)
                        nc.sync.dma_start(out=es[:], in_=es_d[g])
                    if do_mm:
                        ps = ppool.tile([S, N], mybir.dt.float32, tag="ps")
                        for j in range(nch):
                            sl = slice(j * 512, (j + 1) * 512)
                            nc.tensor.matmul(
                                out=ps[:, sl], lhsT=E_sb[:], rhs=X[:, sl],
                                start=True, stop=True,
                            )
            nc.sync.dma_start(out=wout_d[:], in_=X[:])
            nc.sync.dma_start(out=yout_d[:], in_=X[:])
    nc.compile()
    return nc


def _build_fast_program(repeats=1, esbufs=4, xbufs=3, evbufs=3, nA=1, mB=2, aSplit=1,
                        loop_trips=0, psbufs=2, nGp=0):
    """Optimized variant: wide slices (N cols), one [S, N] f32 PSUM tile per
    slice (double-buffered), matmuls in 512-col chunks (PSUM bank limit), and
    the PSUM evacuation + es-multiply split across engines:
      - chunks [0, nA*512): DVE tensor_mul straight from PSUM (1x mode)
      - rest in mB even subgroups: ACT copy PSUM->bf16 SBUF, then DVE
        tensor_mul bf16 x bf16 (2x mode)
    This balances DVE (~(213+FD)/0.96 ns for PSUM-src, (151+FD/2)/0.96 for
    bf16) against ACT (~(172+FD)/1.2), instead of putting both evacuation and
    multiply on DVE at 1x like the baseline."""
    assert N % 512 == 0
    nch = N // 512
    cA = nA * 512
    nB = N - cA
    if mB:
        assert nB % mB == 0 and (nB // mB) % 2 == 0
        sB = nB // mB
    else:
        assert nB == 0
        sB = 0
    cDve = cA - nGp * 512   # trailing nGp 512-regions of A go ACT->GPSIMD
    assert cDve >= 0 and cDve % max(aSplit, 1) == 0
    sA = cDve // aSplit if aSplit else 0

    nc = bacc.Bacc("TRN2", target_bir_lowering=False)
    es_d = nc.dram_tensor("eslices", [G, S, N], mybir.dt.bfloat16, kind="ExternalInput")
    e0_d = nc.dram_tensor("e0", [S, BPC], mybir.dt.bfloat16, kind="ExternalInput")
    ew_d = nc.dram_tensor("ew", [S, S], mybir.dt.bfloat16, kind="ExternalInput")
    wout_d = nc.dram_tensor("wout", [S, N], mybir.dt.bfloat16, kind="ExternalOutput")
    yout_d = nc.dram_tensor("yout", [S, N], mybir.dt.bfloat16, kind="ExternalOutput")

    with TileContext(nc) as tc:
        with (
            tc.tile_pool(name="const", bufs=1) as cpool,
            tc.tile_pool(name="es", bufs=esbufs) as espool,
            tc.tile_pool(name="state", bufs=xbufs) as xpool,
            tc.tile_pool(name="ev", bufs=evbufs) as evpool,
            tc.tile_pool(name="ps", bufs=psbufs, space="PSUM") as ppool,
        ):
            E_sb = cpool.tile([S, S], mybir.dt.bfloat16)
            nc.sync.dma_start(out=E_sb[:], in_=ew_d[:])

            loop = tc.For_i(0, loop_trips) if loop_trips else contextlib.nullcontext()
            with loop:
                for _r in range(repeats):
                    X = xpool.tile([S, N], mybir.dt.bfloat16)
                    nc.vector.memset(X[:], 1.0)

                    for g in range(G):
                        es = espool.tile([S, N], mybir.dt.bfloat16)
                        nc.sync.dma_start(out=es[:], in_=es_d[g])
                        Xn = xpool.tile([S, N], mybir.dt.bfloat16)
                        ps = ppool.tile([S, N], mybir.dt.float32, tag="ps")
                        for j in range(nch):
                            sl = slice(j * 512, (j + 1) * 512)
                            nc.tensor.matmul(
                                out=ps[:, sl], lhsT=E_sb[:], rhs=X[:, sl],
                                start=True, stop=True,
                            )
                        for j in range(aSplit if cDve else 0):
                            sl = slice(j * sA, (j + 1) * sA)
                            nc.vector.tensor_mul(
                                out=Xn[:, sl], in0=ps[:, sl], in1=es[:, sl]
                            )
                        for j in range(nGp):
                            sl = slice(cDve + j * 512, cDve + (j + 1) * 512)
                            gv = evpool.tile([S, 512], mybir.dt.bfloat16, tag=f"gv{j}")
                            nc.scalar.copy(out=gv[:], in_=ps[:, sl])
                            nc.gpsimd.tensor_mul(out=Xn[:, sl], in0=gv[:], in1=es[:, sl])
                        for j in range(mB):
                            sl = slice(cA + j * sB, cA + (j + 1) * sB)
                            ev = evpool.tile([S, sB], mybir.dt.bfloat16, tag=f"ev{j}")
                            nc.scalar.copy(out=ev[:], in_=ps[:, sl])
                            nc.vector.tensor_mul(out=Xn[:, sl], in0=ev[:], in1=es[:, sl])
                        if g == W - 1:
                            nc.sync.dma_start(out=wout_d[:], in_=Xn[:])
                            nc.sync.dma_start(out=Xn[:, 0:BPC], in_=e0_d[:])
                        X = Xn
                    nc.sync.dma_start(out=yout_d[:], in_=X[:])
    nc.compile()
    return nc


LAST_RESULTS = None  # BassKernelResults of the most recent kernel() call (for test harness)


def prep_in_maps(log_observation: np.ndarray, log_transition: np.ndarray):
    """Host-side prep: exp, transpose-to-slice-major, pad, shard per core."""
    E = np.exp(log_transition.astype(np.float64)).astype(np.float32)
    ew_bf = E.astype(bf16)  # lhsT: natural [from, to] layout

    eobs = np.exp(log_observation.astype(np.float32) - BETA).astype(bf16)  # [B,T,S]

    in_maps = []
    for k in range(NCORES):
        blk = eobs[k * BPC:(k + 1) * BPC]          # [BPC, T, S]
        # [S, BPC, T] with time padded: t_pad = t + (W-1); left pad W-1=7, right pad 1
        eT = np.ones((S, BPC, T + W), dtype=bf16)
        eT[:, :, W - 1:W - 1 + T] = blk.transpose(2, 0, 1)
        # slice gather: chain (c,b) at slice g uses t = 64c + g - (W-1) -> t_pad = 64c + g
        st_s, st_b, st_t = eT.strides
        v = np.lib.stride_tricks.as_strided(
            eT, shape=(G, S, C, BPC), strides=(st_t, st_s, L * st_t, st_b)
        )
        es = np.ascontiguousarray(v).reshape(G, S, N)
        e0 = np.ascontiguousarray(eT[:, :, W - 1])   # t=0 column: exp(obs[:,0,:]-BETA)
        in_maps.append({"eslices": es, "e0": e0, "ew": ew_bf})
    return in_maps


def stitch_outputs(results) -> np.ndarray:
    """fp64 host stitch of per-core wout/yout -> [B] -logZ."""
    cnt = np.full(C, G, dtype=np.float64)
    cnt[0] = L + 1        # exact init consumed obs[0] + 64 official steps
    cnt[C - 1] = G - 1    # last chunk consumed one unbiased pad column
    cntw = float(W)
    out = np.empty(B, dtype=np.float64)
    for k in range(NCORES):
        y = results[k]["yout"].astype(np.float64).reshape(S, C, BPC)
        w = results[k]["wout"].astype(np.float64).reshape(S, C, BPC)
        Sy = y.sum(axis=0)            # [C, BPC]
        Sw = w.sum(axis=0)
        ly = np.log(Sy) + (BETA * cnt)[:, None]
        lw = np.log(Sw) + BETA * cntw
        logZ = ly[C - 1] + np.sum(ly[: C - 1] - lw[1:], axis=0)
        out[k * BPC:(k + 1) * BPC] = -logZ
    return out


def kernel(log_observation: np.ndarray, log_transition: np.ndarray) -> np.ndarray:
    assert log_observation.shape == (B, T, S)
    assert log_transition.shape == (S, S)

    in_maps = prep_in_maps(log_observation, log_transition)
    nc = _build_fast_program(**FAST_CFG, psbufs=1 if N >= 4096 else 2)
    res = run_bass_kernel_spmd(nc, in_maps, core_ids=list(range(NCORES)))
    global LAST_RESULTS
    LAST_RESULTS = res
    return stitch_outputs(res.results).astype(np.float32)


if __name__ == "__main__":
    rng = np.random.default_rng(0)
    obs = rng.standard_normal((B, T, S), dtype=np.float32)
    lt = rng.standard_normal((S, S), dtype=np.float32)
    lt = lt - np.log(np.exp(lt).sum(axis=1, keepdims=True))
    print(kernel(obs, lt)[:4])

